# revision 14
# baseline (speedup 1.0000x reference)
"""ApproachLoss kernel for 8 TRN2 NeuronCores (Bass/Tile).

Reference computation (per batch element b):
    deltas[t]  = ||states[b, t+1] - states[b, t]||          t in [0, L-2]
    di[j]      = relu(deltas[j+1] - deltas[j])              j in [0, L-3]
    weighted   = di * reasoning_mask[b, 2:] * approach_weight
    loss       = sum_b sum_j weighted / (sum_b sum_t mask[b, 2:] + 1e-9)

Sharding: pure data-parallel, batch element b -> core b. Each core returns
[weighted_sum_b, mask_sum_b]; the host sums the 16 scalars and divides.

v1 (66 us): f32 upload, PE shift-matmul diffs, all squares on ScalarE.
ScalarE 48 us / TensorE 46 us / DMA 47 us co-bottlenecked.

v3 (this):
 1. states uploaded as bf16 (host cast; the v1 kernel already rounded
    states to bf16 in its DMA - loss rel-err ~1e-4 vs the 2e-2 budget).
    HBM traffic halves to 8.4 MB/core -> DMA floor ~23.5 us. bf16 (not
    fp16) because the DVE 2x packed mode only has uops for bf16:
    measured fp16 tensor_tensor is ~1.6 us/[128,1024] vs ~0.7 for bf16.
 2. Token-group layout: token t lives at partition t//32, free segment
    t%32 (states declared [128, 32*1024]; a pure host-side reshape).
    A 1-token shift is then a free-dim shift of 1024 elements, legal on
    every engine (the BIR verifier rejects partition-offset reads, so
    the window layout would force all diffs through the PE at ~0.5
    us/matmul). 8 chunk DMAs of [128, 4 segments] (1 MB, 8 KB/partition
    lines) stream on the Sync HWDGE ring; all chunks stay resident.
 3. Per chunk: one fused DVE diff over its 3 interior columns
    ([128, 3072], bf16 2x) + one [128, 1024] cross-chunk diff. Square+
    reduce per column j: stt(d*d, accum_out=r[:, j]) on DVE or
    Square+accum on ScalarE - the split is tuned so both engines sit at
    ~DMA-roofline. GpSimd is kept OFF the compute path entirely: its
    TensorTensor is ~2.9 us/[128,1024] and each semaphore op costs
    ~0.5 us there (measured), so any win is eaten by sync overhead.
 4. The 127 partition-boundary deltas (t = 32p+31) are one PE fix:
    ps = U @ x0_seg0 - I @ x7_seg31 (U = shift-up matrix, exact bf16),
    ScalarE square+accum -> r_a[:, 31]. Chunk order [7, 0, 1, .., 6]
    lands both operands early, keeping the fix off the critical tail.
  Tail (all shift-free): rsum = r_d + r_a; E = sqrt(rsum) bf16
  [128, 32]; dmat[:, 0:31] = E[:, 1:32] - E[:, 0:31]; dmat[:, 31] from
  a tiny PE fix (E[p+1, 0] - E[p, 31]); g0 = sum relu(dmat)*mw (mw =
  host-precomputed mask*weight in the same [128, 32] layout); g1 = sum
  mask; GpSimd partition_all_reduce -> DMA out [1, 2].
"""

import numpy as np

B, L, D = 8, 4096, 1024
SEG = 32              # tokens per partition
NCOL = SEG - 1        # diff columns j = 0..30 (+ boundary col 31)
NCHUNK = 8            # 4 segments per chunk
N_CORES = 8

_CACHE = {}


def _bresenham(k, n):
    return {j for j in range(n) if (j + 1) * k // n > j * k // n}


def _config(split_mode):
    """Returns (gp_fused_chunks, bn_chunks, bn_crosses, dve_stt_cols).

    gp_fused_chunks: chunks whose fused 3-col diff runs on GpSimd.
    bn_chunks: chunks whose 3 interior columns are squared via one
        grouped DVE bn_stats; bn_crosses: cross columns squared via
        bn_stats. Remaining columns are squared on ScalarE, except
        dve_stt_cols which use DVE stt.
    """
    if split_mode == "v4":          # GpSimd fused diff experiment: slower
        return {3}, set(), set(), {2, 5, 9, 13, 16, 20, 23, 25, 28, 30}
    # default "v5": GpSimd strictly off the compute path (its activity
    # degrades concurrent DVE/ACT throughput - measured), DVE/ACT square
    # split tuned from measured per-op costs
    return set(), set(), set(), {2, 5, 9, 13, 16, 20, 23, 25, 28, 30}


def _build_nc(split_mode="v3"):
    import concourse.bass as bass  # noqa: F401
    import concourse.tile as tile
    from concourse import bacc, bass_isa, mybir

    f32 = mybir.dt.float32
    bf16 = mybir.dt.bfloat16
    nc = bacc.Bacc(
        "TRN2", target_bir_lowering=False, debug=False, num_devices=N_CORES
    )

    states = nc.declare_dram_parameter(
        "states", [128, SEG * D], bf16, isOutput=False
    )
    cst16 = nc.declare_dram_parameter("cst16", [128, 256], bf16, isOutput=False)
    cst32 = nc.declare_dram_parameter("cst32", [128, 64], f32, isOutput=False)
    out = nc.declare_dram_parameter("out", [1, 2], f32, isOutput=True)

    gp_fused, bn_chunks, bn_crosses, dve_stt_cols = _config(split_mode)
    use_bn = bool(bn_chunks or bn_crosses)

    ADD = mybir.AluOpType.add
    SUB = mybir.AluOpType.subtract
    MUL = mybir.AluOpType.mult
    MAX = mybir.AluOpType.max
    Sq = mybir.ActivationFunctionType.Square
    CPD = 4 * D  # elements per chunk

    with tile.TileContext(nc) as tc:
        with (
            tc.tile_pool(name="consts", bufs=1) as consts,
            tc.tile_pool(name="xpool", bufs=NCHUNK) as xpool,
            tc.tile_pool(name="dfpool", bufs=3) as dfpool,
            tc.tile_pool(name="dxpool", bufs=3) as dxpool,
            tc.tile_pool(name="sqpool", bufs=6) as sqpool,
            tc.tile_pool(name="psum", bufs=1, space="PSUM") as pspool,
        ):
            # consts go FIRST on the Sync ring (tiny, ~0.5 us of stream)
            # so nothing downstream waits on a slow side-channel DMA
            cst16_sb = consts.tile([128, 256], bf16)
            nc.sync.dma_start(out=cst16_sb, in_=cst16[:, :])
            cst32_sb = consts.tile([128, 64], f32)
            nc.sync.dma_start(out=cst32_sb, in_=cst32[:, :])
            U = cst16_sb[:, 0:128]
            nI = cst16_sb[:, 128:256]
            mw_sb = cst32_sb[:, 0:32]
            mask_sb = cst32_sb[:, 32:64]

            r_d = consts.tile([128, SEG], f32)
            nc.vector.memset(r_d, 0.0)
            r_a = consts.tile([128, SEG], f32)
            nc.vector.memset(r_a, 0.0)
            g = consts.tile([128, 2], f32)

            # bn_stats stats tile: column j owns S[:, 12j : 12j+12]
            # (2 groups of 512 elems x 6 stats fields each)
            if use_bn:
                S = consts.tile([128, NCOL * 12], f32)
                nc.vector.memset(S, 0.0)

            def g512(ap, ngroups):  # [128, N] -> [128, ngroups, 512]
                return ap.rearrange("p (g e) -> p g e", e=512)

            xt = {}

            def emit_sq(j, din):
                if j in bn_crosses:
                    nc.vector.bn_stats(
                        S[:, 12 * j : 12 * j + 12], g512(din, 2)
                    )
                elif j in dve_stt_cols:
                    sq = sqpool.tile([128, D], bf16)
                    nc.vector.scalar_tensor_tensor(
                        out=sq, in0=din, scalar=0.0, in1=din, op0=ADD,
                        op1=MUL, accum_out=r_d[:, j : j + 1],
                    )
                else:
                    sq = sqpool.tile([128, D], bf16)
                    nc.scalar.activation(
                        sq, din, Sq, accum_out=r_a[:, j : j + 1]
                    )

            def emit_chunk_ops(c):
                x = xt[c]
                # fused diff over the 3 interior columns 4c..4c+2
                df = dfpool.tile([128, 3 * D], bf16)
                deng = nc.gpsimd if c in gp_fused else nc.vector
                deng.tensor_sub(df, x[:, D : 4 * D], x[:, 0 : 3 * D])
                if c in bn_chunks:
                    nc.vector.bn_stats(
                        S[:, 48 * c : 48 * c + 36], g512(df, 6)
                    )
                else:
                    for k in range(3):
                        emit_sq(4 * c + k, df[:, k * D : (k + 1) * D])
                # cross-chunk diff (col 4c-1) with chunk c-1
                if c >= 1 and (c - 1) in xt:
                    emit_cross(c)
                if (c + 1) in xt:
                    emit_cross(c + 1)

            def emit_cross(c):
                dx = dxpool.tile([128, D], bf16)
                nc.vector.tensor_sub(
                    dx, xt[c][:, 0:D], xt[c - 1][:, 3 * D : 4 * D]
                )
                emit_sq(4 * c - 1, dx)

            order = [NCHUNK - 1] + list(range(NCHUNK - 1))
            for pos, c in enumerate(order):
                x = xpool.tile([128, CPD], bf16)
                # alternate the two DGE paths: even positions on the Sync
                # HWDGE ring, odd on the GpSimd SWDGE ring (the single
                # ring saturates at ~320 GB/s; two rings interleave at
                # packet granularity)
                deng = nc.sync if pos % 2 == 0 else nc.gpsimd
                deng.dma_start(
                    out=x, in_=states[:, CPD * c : CPD * (c + 1)]
                )
                xt[c] = x
                if pos == 0:
                    # dummy sqrt loads the sqrt_and_others ACT table
                    # (contains Square) once. Emitted here - after the
                    # first chunk DMA - so it never head-of-line blocks
                    # the ScalarE queue (it only needs cst32).
                    warm = consts.tile([1, 1], f32)
                    nc.scalar.sqrt(warm, cst32_sb[0:1, 0:1])
                emit_chunk_ops(c)
                if c == 0:
                    # partition-boundary deltas t = 32p+31:
                    # ps[p] = x0[p+1, seg0] - x7[p, seg31]
                    ps = pspool.tile([128, D], f32)
                    for h in range(2):
                        s0, s1 = 512 * h, 512 * (h + 1)
                        nc.tensor.matmul(
                            ps[:, s0:s1], lhsT=U,
                            rhs=xt[0][:, s0:s1],
                            start=True, stop=False,
                        )
                        nc.tensor.matmul(
                            ps[:, s0:s1], lhsT=nI,
                            rhs=xt[NCHUNK - 1][:, 3 * D + s0 : 3 * D + s1],
                            start=False, stop=True,
                        )
                    sqb = sqpool.tile([128, D], bf16)
                    nc.scalar.activation(
                        sqb[0:127, :], ps[0:127, :], Sq,
                        accum_out=r_a[0:127, 31:32],
                    )

            # ---- tail ----
            # mask sum: emitted last so it never blocks the DVE queue
            # (it reads cst32 which arrives early anyway)
            nc.vector.tensor_reduce(
                g[:, 1:2], mask_sb, axis=mybir.AxisListType.X,
                op=mybir.AluOpType.add,
            )
            rsum = consts.tile([128, SEG], f32)
            nc.vector.scalar_tensor_tensor(
                out=rsum, in0=r_d, scalar=0.0, in1=r_a, op0=ADD, op1=ADD
            )
            if use_bn:
                # per 512-group: ssq = M2_even + M2_odd + 256*(me^2 + mo^2)
                def v1(ap):  # [128, N] -> [128, N, 1]
                    return ap.rearrange("p (g o) -> p g o", o=1)

                T = consts.tile([128, NCOL * 12], f32)
                nc.vector.tensor_mul(T, S, S)
                S3 = S[:, :].rearrange("p (g s) -> p g s", s=6)
                T3 = T[:, :].rearrange("p (g s) -> p g s", s=6)
                A = consts.tile([128, 2 * NCOL], f32)
                nc.vector.tensor_add(v1(A[:, :]), S3[:, :, 2:3], S3[:, :, 5:6])
                Bm = consts.tile([128, 2 * NCOL], f32)
                nc.vector.tensor_add(v1(Bm[:, :]), T3[:, :, 1:2], T3[:, :, 4:5])
                G2 = consts.tile([128, 2 * NCOL], f32)
                nc.vector.scalar_tensor_tensor(
                    out=G2, in0=Bm, scalar=256.0, in1=A, op0=MUL, op1=ADD
                )
                rbn = consts.tile([128, SEG], f32)
                nc.vector.memset(rbn, 0.0)
                V = G2[:, :].rearrange("p (c t) -> p c t", t=2)
                nc.vector.tensor_add(
                    v1(rbn[:, 0:NCOL]), V[:, :, 0:1], V[:, :, 1:2]
                )
                nc.vector.scalar_tensor_tensor(
                    out=rsum, in0=rsum, scalar=0.0, in1=rbn, op0=ADD, op1=ADD
                )
            e_sb = consts.tile([128, SEG], bf16)
            nc.scalar.activation(e_sb, rsum, mybir.ActivationFunctionType.Sqrt)

            dmat = consts.tile([128, SEG], bf16)
            nc.vector.tensor_sub(
                dmat[:, 0:NCOL], e_sb[:, 1:SEG], e_sb[:, 0:NCOL]
            )
            # boundary di: dmat[p, 31] = E[p+1, 0] - E[p, 31]
            ps2 = pspool.tile([128, 1], f32)
            nc.tensor.matmul(
                ps2, lhsT=U, rhs=e_sb[:, 0:1], start=True, stop=False
            )
            nc.tensor.matmul(
                ps2, lhsT=nI, rhs=e_sb[:, 31:32], start=False, stop=True
            )
            nc.vector.tensor_copy(dmat[:, 31:32], ps2)

            wt = consts.tile([128, SEG], f32)
            nc.vector.scalar_tensor_tensor(
                out=wt,
                in0=dmat,
                scalar=0.0,
                in1=mw_sb,
                op0=MAX,
                op1=MUL,
                accum_out=g[:, 0:1],
            )

            red = consts.tile([128, 2], f32)
            nc.gpsimd.partition_all_reduce(
                red, g, channels=128, reduce_op=bass_isa.ReduceOp.add
            )
            nc.sync.dma_start(out=out[:, :], in_=red[0:1, :])

    nc.compile()
    return nc


def _host_consts():
    import ml_dtypes

    cst16 = np.zeros((128, 256), dtype=ml_dtypes.bfloat16)
    for p in range(127):
        cst16[p + 1, p] = 1.0          # U[p, i] = 1 iff p == i+1
    for p in range(128):
        cst16[p, 128 + p] = -1.0       # -I
    return cst16


def _per_core_inputs(states_b, mask_b, rp_b, cst16):
    import ml_dtypes

    # weight coefficients: mw[p, j] = mask[t+2] * weight[t] at t = 32p+j
    t = np.arange(L - 2, dtype=np.float64)
    dist = np.maximum(float(rp_b) - t - 2.0, 0.0)
    weight = np.where(dist < 5, 2.0 + (5.0 - dist) * 0.5, 1.0).astype(np.float32)
    mwvec = (mask_b[2:L] * weight).astype(np.float32)  # [L-2]
    vals = np.zeros(L, dtype=np.float32)
    vals[: L - 2] = mwvec
    mw = vals.reshape(128, SEG)

    mt = mask_b.astype(np.float32).copy()
    mt[0:2] = 0.0
    maskt = mt.reshape(128, SEG)

    cst32 = np.concatenate([mw, maskt], axis=1)  # [128, 64]

    return {
        "states": np.ascontiguousarray(
            states_b.astype(ml_dtypes.bfloat16).reshape(128, SEG * D)
        ),
        "cst16": cst16,
        "cst32": np.ascontiguousarray(cst32),
    }


def _get_nc(split_mode="v3"):
    key = ("nc", split_mode)
    if key not in _CACHE:
        _CACHE[key] = _build_nc(split_mode)
    return _CACHE[key]


def _run(states, reasoning_mask, result_token_positions, trace=False,
         split_mode="v3"):
    from concourse.bass_utils import run_bass_kernel_spmd

    states = np.asarray(states, dtype=np.float32)
    mask = np.asarray(reasoning_mask, dtype=np.float32)
    rp = np.asarray(result_token_positions)

    cst16 = _host_consts()
    in_maps = [
        _per_core_inputs(states[b], mask[b], rp[b], cst16)
        for b in range(N_CORES)
    ]
    nc = _get_nc(split_mode)
    res = run_bass_kernel_spmd(
        nc, in_maps, core_ids=list(range(N_CORES)), trace=trace
    )
    partials = np.stack([res.results[i]["out"][0] for i in range(N_CORES)])  # [8, 2]
    s = partials[:, 0].astype(np.float64).sum()
    m = partials[:, 1].astype(np.float64).sum()
    value = np.float32(s / (m + 1e-9))
    return value, res


def kernel(states, reasoning_mask, result_token_positions):
    value, _ = _run(states, reasoning_mask, result_token_positions)
    return np.asarray(value, dtype=np.float32)


# revision 15
# speedup vs baseline: 1.1403x; 1.1403x over previous
"""ApproachLoss kernel for 8 TRN2 NeuronCores (Bass/Tile).

Reference computation (per batch element b):
    deltas[t]  = ||states[b, t+1] - states[b, t]||          t in [0, L-2]
    di[j]      = relu(deltas[j+1] - deltas[j])              j in [0, L-3]
    weighted   = di * reasoning_mask[b, 2:] * approach_weight
    loss       = sum_b sum_j weighted / (sum_b sum_t mask[b, 2:] + 1e-9)

Sharding: pure data-parallel, batch element b -> core b. Each core returns
[weighted_sum_b, mask_sum_b]; the host sums the 16 scalars and divides.

v1 (66 us): f32 upload, PE shift-matmul diffs, all squares on ScalarE.
ScalarE 48 us / TensorE 46 us / DMA 47 us co-bottlenecked.

v3 (this):
 1. states uploaded as bf16 (host cast; the v1 kernel already rounded
    states to bf16 in its DMA - loss rel-err ~1e-4 vs the 2e-2 budget).
    HBM traffic halves to 8.4 MB/core -> DMA floor ~23.5 us. bf16 (not
    fp16) because the DVE 2x packed mode only has uops for bf16:
    measured fp16 tensor_tensor is ~1.6 us/[128,1024] vs ~0.7 for bf16.
 2. Token-group layout: token t lives at partition t//32, free segment
    t%32 (states declared [128, 32*1024]; a pure host-side reshape).
    A 1-token shift is then a free-dim shift of 1024 elements, legal on
    every engine (the BIR verifier rejects partition-offset reads, so
    the window layout would force all diffs through the PE at ~0.5
    us/matmul). 8 chunk DMAs of [128, 4 segments] (1 MB, 8 KB/partition
    lines) stream on the Sync HWDGE ring; all chunks stay resident.
 3. Per chunk: one fused DVE diff over its 3 interior columns
    ([128, 3072], bf16 2x) + one [128, 1024] cross-chunk diff. Square+
    reduce per column j: stt(d*d, accum_out=r[:, j]) on DVE or
    Square+accum on ScalarE - the split is tuned so both engines sit at
    ~DMA-roofline. GpSimd is kept OFF the compute path entirely: its
    TensorTensor is ~2.9 us/[128,1024] and each semaphore op costs
    ~0.5 us there (measured), so any win is eaten by sync overhead.
 4. The 127 partition-boundary deltas (t = 32p+31) are one PE fix:
    ps = U @ x0_seg0 - I @ x7_seg31 (U = shift-up matrix, exact bf16),
    ScalarE square+accum -> r_a[:, 31]. Chunk order [7, 0, 1, .., 6]
    lands both operands early, keeping the fix off the critical tail.
  Tail (all shift-free): rsum = r_d + r_a; E = sqrt(rsum) bf16
  [128, 32]; dmat[:, 0:31] = E[:, 1:32] - E[:, 0:31]; dmat[:, 31] from
  a tiny PE fix (E[p+1, 0] - E[p, 31]); g0 = sum relu(dmat)*mw (mw =
  host-precomputed mask*weight in the same [128, 32] layout); g1 = sum
  mask; GpSimd partition_all_reduce -> DMA out [1, 2].
"""

import numpy as np

B, L, D = 8, 4096, 1024
SEG = 32              # tokens per partition
NCOL = SEG - 1        # diff columns j = 0..30 (+ boundary col 31)
NCHUNK = 8            # 4 segments per chunk
N_CORES = 8

_CACHE = {}


def _bresenham(k, n):
    return {j for j in range(n) if (j + 1) * k // n > j * k // n}


def _config(split_mode):
    """Returns (gp_fused_chunks, bn_chunks, bn_crosses, dve_stt_cols).

    gp_fused_chunks: chunks whose fused 3-col diff runs on GpSimd.
    bn_chunks: chunks whose 3 interior columns are squared via one
        grouped DVE bn_stats; bn_crosses: cross columns squared via
        bn_stats. Remaining columns are squared on ScalarE, except
        dve_stt_cols which use DVE stt.
    """
    if split_mode == "v4":          # GpSimd fused diff experiment: slower
        return {3}, set(), set(), {2, 5, 9, 13, 16, 20, 23, 25, 28, 30}
    # default "v5": GpSimd strictly off the compute path (its activity
    # degrades concurrent DVE/ACT throughput - measured), DVE/ACT square
    # split tuned from measured per-op costs
    return set(), set(), set(), {2, 5, 9, 13, 16, 20, 23, 25, 28, 30}


def _build_nc(split_mode="v3"):
    import concourse.bass as bass  # noqa: F401
    import concourse.tile as tile
    from concourse import bacc, bass_isa, mybir

    f32 = mybir.dt.float32
    bf16 = mybir.dt.bfloat16
    nc = bacc.Bacc(
        "TRN2", target_bir_lowering=False, debug=False, num_devices=N_CORES
    )

    states = nc.declare_dram_parameter(
        "states", [128, SEG * D], bf16, isOutput=False
    )
    cst16 = nc.declare_dram_parameter("cst16", [128, 256], bf16, isOutput=False)
    cst32 = nc.declare_dram_parameter("cst32", [128, 64], f32, isOutput=False)
    out = nc.declare_dram_parameter("out", [1, 2], f32, isOutput=True)

    gp_fused, bn_chunks, bn_crosses, dve_stt_cols = _config(split_mode)
    use_bn = bool(bn_chunks or bn_crosses)

    ADD = mybir.AluOpType.add
    SUB = mybir.AluOpType.subtract
    MUL = mybir.AluOpType.mult
    MAX = mybir.AluOpType.max
    Sq = mybir.ActivationFunctionType.Square
    CPD = 4 * D  # elements per chunk

    with tile.TileContext(nc) as tc:
        with (
            tc.tile_pool(name="consts", bufs=1) as consts,
            tc.tile_pool(name="xpool", bufs=NCHUNK) as xpool,
            tc.tile_pool(name="dfpool", bufs=3) as dfpool,
            tc.tile_pool(name="dxpool", bufs=3) as dxpool,
            tc.tile_pool(name="sqpool", bufs=6) as sqpool,
            tc.tile_pool(name="psum", bufs=1, space="PSUM") as pspool,
        ):
            # consts go FIRST on the Sync ring (tiny, ~0.5 us of stream)
            # so nothing downstream waits on a slow side-channel DMA
            cst16_sb = consts.tile([128, 256], bf16)
            nc.sync.dma_start(out=cst16_sb, in_=cst16[:, :])
            cst32_sb = consts.tile([128, 64], f32)
            nc.sync.dma_start(out=cst32_sb, in_=cst32[:, :])
            U = cst16_sb[:, 0:128]
            nI = cst16_sb[:, 128:256]
            mw_sb = cst32_sb[:, 0:32]
            mask_sb = cst32_sb[:, 32:64]

            r_d = consts.tile([128, SEG], f32)
            nc.vector.memset(r_d, 0.0)
            r_a = consts.tile([128, SEG], f32)
            nc.vector.memset(r_a, 0.0)
            g = consts.tile([128, 2], f32)

            # bn_stats stats tile: column j owns S[:, 12j : 12j+12]
            # (2 groups of 512 elems x 6 stats fields each)
            if use_bn:
                S = consts.tile([128, NCOL * 12], f32)
                nc.vector.memset(S, 0.0)

            def g512(ap, ngroups):  # [128, N] -> [128, ngroups, 512]
                return ap.rearrange("p (g e) -> p g e", e=512)

            xt = {}

            def emit_sq(j, din):
                if j in bn_crosses:
                    nc.vector.bn_stats(
                        S[:, 12 * j : 12 * j + 12], g512(din, 2)
                    )
                elif j in dve_stt_cols:
                    sq = sqpool.tile([128, D], bf16)
                    nc.vector.scalar_tensor_tensor(
                        out=sq, in0=din, scalar=0.0, in1=din, op0=ADD,
                        op1=MUL, accum_out=r_d[:, j : j + 1],
                    )
                else:
                    sq = sqpool.tile([128, D], bf16)
                    nc.scalar.activation(
                        sq, din, Sq, accum_out=r_a[:, j : j + 1]
                    )

            def emit_chunk_ops(c):
                x = xt[c]
                # fused diff over the 3 interior columns 4c..4c+2
                df = dfpool.tile([128, 3 * D], bf16)
                deng = nc.gpsimd if c in gp_fused else nc.vector
                deng.tensor_sub(df, x[:, D : 4 * D], x[:, 0 : 3 * D])
                if c in bn_chunks:
                    nc.vector.bn_stats(
                        S[:, 48 * c : 48 * c + 36], g512(df, 6)
                    )
                else:
                    for k in range(3):
                        emit_sq(4 * c + k, df[:, k * D : (k + 1) * D])
                # cross-chunk diff (col 4c-1) with chunk c-1
                if c >= 1 and (c - 1) in xt:
                    emit_cross(c)
                if (c + 1) in xt:
                    emit_cross(c + 1)

            def emit_cross(c):
                dx = dxpool.tile([128, D], bf16)
                nc.vector.tensor_sub(
                    dx, xt[c][:, 0:D], xt[c - 1][:, 3 * D : 4 * D]
                )
                emit_sq(4 * c - 1, dx)

            order = [NCHUNK - 1] + list(range(NCHUNK - 1))
            for pos, c in enumerate(order):
                x = xpool.tile([128, CPD], bf16)
                nc.sync.dma_start(
                    out=x, in_=states[:, CPD * c : CPD * (c + 1)]
                )
                xt[c] = x
                if pos == 0:
                    # dummy sqrt loads the sqrt_and_others ACT table
                    # (contains Square) once. Emitted here - after the
                    # first chunk DMA - so it never head-of-line blocks
                    # the ScalarE queue (it only needs cst32).
                    warm = consts.tile([1, 1], f32)
                    nc.scalar.sqrt(warm, cst32_sb[0:1, 0:1])
                emit_chunk_ops(c)
                if c == 0:
                    # partition-boundary deltas t = 32p+31:
                    # ps[p] = x0[p+1, seg0] - x7[p, seg31]
                    ps = pspool.tile([128, D], f32)
                    for h in range(2):
                        s0, s1 = 512 * h, 512 * (h + 1)
                        nc.tensor.matmul(
                            ps[:, s0:s1], lhsT=U,
                            rhs=xt[0][:, s0:s1],
                            start=True, stop=False,
                        )
                        nc.tensor.matmul(
                            ps[:, s0:s1], lhsT=nI,
                            rhs=xt[NCHUNK - 1][:, 3 * D + s0 : 3 * D + s1],
                            start=False, stop=True,
                        )
                    sqb = sqpool.tile([128, D], bf16)
                    nc.scalar.activation(
                        sqb[0:127, :], ps[0:127, :], Sq,
                        accum_out=r_a[0:127, 31:32],
                    )

            # ---- tail ----
            # mask sum: emitted last so it never blocks the DVE queue
            # (it reads cst32 which arrives early anyway)
            nc.vector.tensor_reduce(
                g[:, 1:2], mask_sb, axis=mybir.AxisListType.X,
                op=mybir.AluOpType.add,
            )
            rsum = consts.tile([128, SEG], f32)
            nc.vector.scalar_tensor_tensor(
                out=rsum, in0=r_d, scalar=0.0, in1=r_a, op0=ADD, op1=ADD
            )
            if use_bn:
                # per 512-group: ssq = M2_even + M2_odd + 256*(me^2 + mo^2)
                def v1(ap):  # [128, N] -> [128, N, 1]
                    return ap.rearrange("p (g o) -> p g o", o=1)

                T = consts.tile([128, NCOL * 12], f32)
                nc.vector.tensor_mul(T, S, S)
                S3 = S[:, :].rearrange("p (g s) -> p g s", s=6)
                T3 = T[:, :].rearrange("p (g s) -> p g s", s=6)
                A = consts.tile([128, 2 * NCOL], f32)
                nc.vector.tensor_add(v1(A[:, :]), S3[:, :, 2:3], S3[:, :, 5:6])
                Bm = consts.tile([128, 2 * NCOL], f32)
                nc.vector.tensor_add(v1(Bm[:, :]), T3[:, :, 1:2], T3[:, :, 4:5])
                G2 = consts.tile([128, 2 * NCOL], f32)
                nc.vector.scalar_tensor_tensor(
                    out=G2, in0=Bm, scalar=256.0, in1=A, op0=MUL, op1=ADD
                )
                rbn = consts.tile([128, SEG], f32)
                nc.vector.memset(rbn, 0.0)
                V = G2[:, :].rearrange("p (c t) -> p c t", t=2)
                nc.vector.tensor_add(
                    v1(rbn[:, 0:NCOL]), V[:, :, 0:1], V[:, :, 1:2]
                )
                nc.vector.scalar_tensor_tensor(
                    out=rsum, in0=rsum, scalar=0.0, in1=rbn, op0=ADD, op1=ADD
                )
            e_sb = consts.tile([128, SEG], bf16)
            nc.scalar.activation(e_sb, rsum, mybir.ActivationFunctionType.Sqrt)

            dmat = consts.tile([128, SEG], bf16)
            nc.vector.tensor_sub(
                dmat[:, 0:NCOL], e_sb[:, 1:SEG], e_sb[:, 0:NCOL]
            )
            # boundary di: dmat[p, 31] = E[p+1, 0] - E[p, 31]
            ps2 = pspool.tile([128, 1], f32)
            nc.tensor.matmul(
                ps2, lhsT=U, rhs=e_sb[:, 0:1], start=True, stop=False
            )
            nc.tensor.matmul(
                ps2, lhsT=nI, rhs=e_sb[:, 31:32], start=False, stop=True
            )
            nc.vector.tensor_copy(dmat[:, 31:32], ps2)

            wt = consts.tile([128, SEG], f32)
            nc.vector.scalar_tensor_tensor(
                out=wt,
                in0=dmat,
                scalar=0.0,
                in1=mw_sb,
                op0=MAX,
                op1=MUL,
                accum_out=g[:, 0:1],
            )

            red = consts.tile([128, 2], f32)
            nc.gpsimd.partition_all_reduce(
                red, g, channels=128, reduce_op=bass_isa.ReduceOp.add
            )
            nc.sync.dma_start(out=out[:, :], in_=red[0:1, :])

    nc.compile()
    return nc


def _host_consts():
    import ml_dtypes

    cst16 = np.zeros((128, 256), dtype=ml_dtypes.bfloat16)
    for p in range(127):
        cst16[p + 1, p] = 1.0          # U[p, i] = 1 iff p == i+1
    for p in range(128):
        cst16[p, 128 + p] = -1.0       # -I
    return cst16


def _per_core_inputs(states_b, mask_b, rp_b, cst16):
    import ml_dtypes

    # weight coefficients: mw[p, j] = mask[t+2] * weight[t] at t = 32p+j
    t = np.arange(L - 2, dtype=np.float64)
    dist = np.maximum(float(rp_b) - t - 2.0, 0.0)
    weight = np.where(dist < 5, 2.0 + (5.0 - dist) * 0.5, 1.0).astype(np.float32)
    mwvec = (mask_b[2:L] * weight).astype(np.float32)  # [L-2]
    vals = np.zeros(L, dtype=np.float32)
    vals[: L - 2] = mwvec
    mw = vals.reshape(128, SEG)

    mt = mask_b.astype(np.float32).copy()
    mt[0:2] = 0.0
    maskt = mt.reshape(128, SEG)

    cst32 = np.concatenate([mw, maskt], axis=1)  # [128, 64]

    return {
        "states": np.ascontiguousarray(
            states_b.astype(ml_dtypes.bfloat16).reshape(128, SEG * D)
        ),
        "cst16": cst16,
        "cst32": np.ascontiguousarray(cst32),
    }


def _get_nc(split_mode="v3"):
    key = ("nc", split_mode)
    if key not in _CACHE:
        _CACHE[key] = _build_nc(split_mode)
    return _CACHE[key]


def _run(states, reasoning_mask, result_token_positions, trace=False,
         split_mode="v3"):
    from concourse.bass_utils import run_bass_kernel_spmd

    states = np.asarray(states, dtype=np.float32)
    mask = np.asarray(reasoning_mask, dtype=np.float32)
    rp = np.asarray(result_token_positions)

    cst16 = _host_consts()
    in_maps = [
        _per_core_inputs(states[b], mask[b], rp[b], cst16)
        for b in range(N_CORES)
    ]
    nc = _get_nc(split_mode)
    res = run_bass_kernel_spmd(
        nc, in_maps, core_ids=list(range(N_CORES)), trace=trace
    )
    partials = np.stack([res.results[i]["out"][0] for i in range(N_CORES)])  # [8, 2]
    s = partials[:, 0].astype(np.float64).sum()
    m = partials[:, 1].astype(np.float64).sum()
    value = np.float32(s / (m + 1e-9))
    return value, res


def kernel(states, reasoning_mask, result_token_positions):
    value, _ = _run(states, reasoning_mask, result_token_positions)
    return np.asarray(value, dtype=np.float32)


# revision 17
# speedup vs baseline: 1.2986x; 1.1388x over previous
"""ApproachLoss kernel for 8 TRN2 NeuronCores (Bass/Tile).

Reference computation (per batch element b):
    deltas[t]  = ||states[b, t+1] - states[b, t]||          t in [0, L-2]
    di[j]      = relu(deltas[j+1] - deltas[j])              j in [0, L-3]
    weighted   = di * reasoning_mask[b, 2:] * approach_weight
    loss       = sum_b sum_j weighted / (sum_b sum_t mask[b, 2:] + 1e-9)

Sharding: pure data-parallel, batch element b -> core b. Each core returns
[weighted_sum_b, mask_sum_b]; the host sums the 16 scalars and divides.

v1 (66 us): f32 upload, PE shift-matmul diffs, all squares on ScalarE.
ScalarE 48 us / TensorE 46 us / DMA 47 us co-bottlenecked.

v3 (this):
 1. states uploaded as bf16 (host cast; the v1 kernel already rounded
    states to bf16 in its DMA - loss rel-err ~1e-4 vs the 2e-2 budget).
    HBM traffic halves to 8.4 MB/core -> DMA floor ~23.5 us. bf16 (not
    fp16) because the DVE 2x packed mode only has uops for bf16:
    measured fp16 tensor_tensor is ~1.6 us/[128,1024] vs ~0.7 for bf16.
 2. Token-group layout: token t lives at partition t//32, free segment
    t%32 (states declared [128, 32*1024]; a pure host-side reshape).
    A 1-token shift is then a free-dim shift of 1024 elements, legal on
    every engine (the BIR verifier rejects partition-offset reads, so
    the window layout would force all diffs through the PE at ~0.5
    us/matmul). 8 chunk DMAs of [128, 4 segments] (1 MB, 8 KB/partition
    lines) stream on the Sync HWDGE ring; all chunks stay resident.
 3. Per chunk: one fused DVE diff over its 3 interior columns
    ([128, 3072], bf16 2x) + one [128, 1024] cross-chunk diff. Square+
    reduce per column j: stt(d*d, accum_out=r[:, j]) on DVE or
    Square+accum on ScalarE - the split is tuned so both engines sit at
    ~DMA-roofline. GpSimd is kept OFF the compute path entirely: its
    TensorTensor is ~2.9 us/[128,1024] and each semaphore op costs
    ~0.5 us there (measured), so any win is eaten by sync overhead.
 4. The 127 partition-boundary deltas (t = 32p+31) are one PE fix:
    ps = U @ x0_seg0 - I @ x7_seg31 (U = shift-up matrix, exact bf16),
    ScalarE square+accum -> r_a[:, 31]. Chunk order [7, 0, 1, .., 6]
    lands both operands early, keeping the fix off the critical tail.
  Tail (all shift-free): rsum = r_d + r_a; E = sqrt(rsum) bf16
  [128, 32]; dmat[:, 0:31] = E[:, 1:32] - E[:, 0:31]; dmat[:, 31] from
  a tiny PE fix (E[p+1, 0] - E[p, 31]); g0 = sum relu(dmat)*mw (mw =
  host-precomputed mask*weight in the same [128, 32] layout); g1 = sum
  mask; GpSimd partition_all_reduce -> DMA out [1, 2].
"""

import numpy as np

B, L, D = 8, 4096, 1024
SEG = 32              # tokens per partition
NCOL = SEG - 1        # diff columns j = 0..30 (+ boundary col 31)
NCHUNK = 8            # 4 segments per chunk
N_CORES = 8

_CACHE = {}


def _bresenham(k, n):
    return {j for j in range(n) if (j + 1) * k // n > j * k // n}


def _config(split_mode):
    """Returns (gp_fused_chunks, bn_chunks, bn_crosses, dve_stt_cols).

    gp_fused_chunks: chunks whose fused 3-col diff runs on GpSimd.
    bn_chunks: chunks whose 3 interior columns are squared via one
        grouped DVE bn_stats; bn_crosses: cross columns squared via
        bn_stats. Remaining columns are squared on ScalarE, except
        dve_stt_cols which use DVE stt.
    """
    if split_mode == "v4":          # GpSimd fused diff experiment: slower
        return {3}, set(), set(), {2, 5, 9, 13, 16, 20, 23, 25, 28, 30}
    # default "v5": GpSimd strictly off the compute path (its activity
    # degrades concurrent DVE/ACT throughput - measured), DVE/ACT square
    # split tuned from measured per-op costs
    return set(), set(), set(), {2, 5, 9, 13, 16, 20, 23, 25, 28, 30}


def _build_nc(split_mode="v3"):
    import concourse.bass as bass  # noqa: F401
    import concourse.tile as tile
    from concourse import bacc, bass_isa, mybir

    f32 = mybir.dt.float32
    bf16 = mybir.dt.bfloat16
    nc = bacc.Bacc(
        "TRN2", target_bir_lowering=False, debug=False, num_devices=N_CORES
    )

    states = nc.declare_dram_parameter(
        "states", [128, SEG * D], bf16, isOutput=False
    )
    cst16 = nc.declare_dram_parameter("cst16", [128, 256], bf16, isOutput=False)
    cst32 = nc.declare_dram_parameter("cst32", [128, 64], f32, isOutput=False)
    out = nc.declare_dram_parameter("out", [1, 2], f32, isOutput=True)

    gp_fused, bn_chunks, bn_crosses, dve_stt_cols = _config(split_mode)
    use_bn = bool(bn_chunks or bn_crosses)

    ADD = mybir.AluOpType.add
    SUB = mybir.AluOpType.subtract
    MUL = mybir.AluOpType.mult
    MAX = mybir.AluOpType.max
    Sq = mybir.ActivationFunctionType.Square
    CPD = 4 * D  # elements per chunk

    with tile.TileContext(nc) as tc:
        with (
            tc.tile_pool(name="consts", bufs=1) as consts,
            tc.tile_pool(name="xpool", bufs=NCHUNK) as xpool,
            tc.tile_pool(name="dfpool", bufs=3) as dfpool,
            tc.tile_pool(name="dxpool", bufs=3) as dxpool,
            tc.tile_pool(name="sqpool", bufs=6) as sqpool,
            tc.tile_pool(name="psum", bufs=1, space="PSUM") as pspool,
        ):
            cst16_sb = consts.tile([128, 256], bf16)
            cst32_sb = consts.tile([128, 64], f32)
            U = cst16_sb[:, 0:128]
            nI = cst16_sb[:, 128:256]
            mw_sb = cst32_sb[:, 0:32]
            mask_sb = cst32_sb[:, 32:64]

            r_d = consts.tile([128, SEG], f32)
            nc.vector.memset(r_d, 0.0)
            r_a = consts.tile([128, SEG], f32)
            nc.vector.memset(r_a, 0.0)
            g = consts.tile([128, 2], f32)

            # bn_stats stats tile: column j owns S[:, 12j : 12j+12]
            # (2 groups of 512 elems x 6 stats fields each)
            if use_bn:
                S = consts.tile([128, NCOL * 12], f32)
                nc.vector.memset(S, 0.0)

            def g512(ap, ngroups):  # [128, N] -> [128, ngroups, 512]
                return ap.rearrange("p (g e) -> p g e", e=512)

            xt = {}

            def emit_sq(j, din):
                if j in bn_crosses:
                    nc.vector.bn_stats(
                        S[:, 12 * j : 12 * j + 12], g512(din, 2)
                    )
                elif j in dve_stt_cols:
                    sq = sqpool.tile([128, D], bf16)
                    nc.vector.scalar_tensor_tensor(
                        out=sq, in0=din, scalar=0.0, in1=din, op0=ADD,
                        op1=MUL, accum_out=r_d[:, j : j + 1],
                    )
                else:
                    sq = sqpool.tile([128, D], bf16)
                    nc.scalar.activation(
                        sq, din, Sq, accum_out=r_a[:, j : j + 1]
                    )

            def emit_chunk_ops(c):
                x = xt[c]
                # fused diff over the 3 interior columns 4c..4c+2
                df = dfpool.tile([128, 3 * D], bf16)
                deng = nc.gpsimd if c in gp_fused else nc.vector
                deng.tensor_sub(df, x[:, D : 4 * D], x[:, 0 : 3 * D])
                if c in bn_chunks:
                    nc.vector.bn_stats(
                        S[:, 48 * c : 48 * c + 36], g512(df, 6)
                    )
                else:
                    for k in range(3):
                        emit_sq(4 * c + k, df[:, k * D : (k + 1) * D])
                # cross-chunk diff (col 4c-1) with chunk c-1
                if c >= 1 and (c - 1) in xt:
                    emit_cross(c)
                if (c + 1) in xt:
                    emit_cross(c + 1)

            def emit_cross(c):
                dx = dxpool.tile([128, D], bf16)
                nc.vector.tensor_sub(
                    dx, xt[c][:, 0:D], xt[c - 1][:, 3 * D : 4 * D]
                )
                emit_sq(4 * c - 1, dx)

            order = [NCHUNK - 1] + list(range(NCHUNK - 1))
            for pos, c in enumerate(order):
                x = xpool.tile([128, CPD], bf16)
                nc.sync.dma_start(
                    out=x, in_=states[:, CPD * c : CPD * (c + 1)]
                )
                xt[c] = x
                if pos == 1:
                    # consts ride the Sync ring behind the first two
                    # chunks (tiny; needed only by the boundary fix and
                    # the tail). No warm-up sqrt: the first Square loads
                    # the shared sqrt_and_others ACT table itself.
                    nc.sync.dma_start(out=cst16_sb, in_=cst16[:, :])
                    nc.sync.dma_start(out=cst32_sb, in_=cst32[:, :])
                emit_chunk_ops(c)
                if c == 0:
                    # partition-boundary deltas t = 32p+31:
                    # ps[p] = x0[p+1, seg0] - x7[p, seg31]
                    ps = pspool.tile([128, D], f32)
                    for h in range(2):
                        s0, s1 = 512 * h, 512 * (h + 1)
                        nc.tensor.matmul(
                            ps[:, s0:s1], lhsT=U,
                            rhs=xt[0][:, s0:s1],
                            start=True, stop=False,
                        )
                        nc.tensor.matmul(
                            ps[:, s0:s1], lhsT=nI,
                            rhs=xt[NCHUNK - 1][:, 3 * D + s0 : 3 * D + s1],
                            start=False, stop=True,
                        )
                    sqb = sqpool.tile([128, D], bf16)
                    nc.scalar.activation(
                        sqb[0:127, :], ps[0:127, :], Sq,
                        accum_out=r_a[0:127, 31:32],
                    )

            # ---- tail ----
            # mask sum: emitted last so it never blocks the DVE queue
            # (it reads cst32 which arrives early anyway)
            nc.vector.tensor_reduce(
                g[:, 1:2], mask_sb, axis=mybir.AxisListType.X,
                op=mybir.AluOpType.add,
            )
            rsum = consts.tile([128, SEG], f32)
            nc.vector.scalar_tensor_tensor(
                out=rsum, in0=r_d, scalar=0.0, in1=r_a, op0=ADD, op1=ADD
            )
            if use_bn:
                # per 512-group: ssq = M2_even + M2_odd + 256*(me^2 + mo^2)
                def v1(ap):  # [128, N] -> [128, N, 1]
                    return ap.rearrange("p (g o) -> p g o", o=1)

                T = consts.tile([128, NCOL * 12], f32)
                nc.vector.tensor_mul(T, S, S)
                S3 = S[:, :].rearrange("p (g s) -> p g s", s=6)
                T3 = T[:, :].rearrange("p (g s) -> p g s", s=6)
                A = consts.tile([128, 2 * NCOL], f32)
                nc.vector.tensor_add(v1(A[:, :]), S3[:, :, 2:3], S3[:, :, 5:6])
                Bm = consts.tile([128, 2 * NCOL], f32)
                nc.vector.tensor_add(v1(Bm[:, :]), T3[:, :, 1:2], T3[:, :, 4:5])
                G2 = consts.tile([128, 2 * NCOL], f32)
                nc.vector.scalar_tensor_tensor(
                    out=G2, in0=Bm, scalar=256.0, in1=A, op0=MUL, op1=ADD
                )
                rbn = consts.tile([128, SEG], f32)
                nc.vector.memset(rbn, 0.0)
                V = G2[:, :].rearrange("p (c t) -> p c t", t=2)
                nc.vector.tensor_add(
                    v1(rbn[:, 0:NCOL]), V[:, :, 0:1], V[:, :, 1:2]
                )
                nc.vector.scalar_tensor_tensor(
                    out=rsum, in0=rsum, scalar=0.0, in1=rbn, op0=ADD, op1=ADD
                )
            e_sb = consts.tile([128, SEG], bf16)
            nc.scalar.activation(e_sb, rsum, mybir.ActivationFunctionType.Sqrt)

            dmat = consts.tile([128, SEG], bf16)
            nc.vector.tensor_sub(
                dmat[:, 0:NCOL], e_sb[:, 1:SEG], e_sb[:, 0:NCOL]
            )
            # boundary di: dmat[p, 31] = E[p+1, 0] - E[p, 31]
            ps2 = pspool.tile([128, 1], f32)
            nc.tensor.matmul(
                ps2, lhsT=U, rhs=e_sb[:, 0:1], start=True, stop=False
            )
            nc.tensor.matmul(
                ps2, lhsT=nI, rhs=e_sb[:, 31:32], start=False, stop=True
            )
            nc.vector.tensor_copy(dmat[:, 31:32], ps2)

            wt = consts.tile([128, SEG], f32)
            nc.vector.scalar_tensor_tensor(
                out=wt,
                in0=dmat,
                scalar=0.0,
                in1=mw_sb,
                op0=MAX,
                op1=MUL,
                accum_out=g[:, 0:1],
            )

            red = consts.tile([128, 2], f32)
            nc.gpsimd.partition_all_reduce(
                red, g, channels=128, reduce_op=bass_isa.ReduceOp.add
            )
            nc.sync.dma_start(out=out[:, :], in_=red[0:1, :])

    nc.compile()
    return nc


def _host_consts():
    import ml_dtypes

    cst16 = np.zeros((128, 256), dtype=ml_dtypes.bfloat16)
    for p in range(127):
        cst16[p + 1, p] = 1.0          # U[p, i] = 1 iff p == i+1
    for p in range(128):
        cst16[p, 128 + p] = -1.0       # -I
    return cst16


def _per_core_inputs(states_b, mask_b, rp_b, cst16):
    import ml_dtypes

    # weight coefficients: mw[p, j] = mask[t+2] * weight[t] at t = 32p+j
    t = np.arange(L - 2, dtype=np.float64)
    dist = np.maximum(float(rp_b) - t - 2.0, 0.0)
    weight = np.where(dist < 5, 2.0 + (5.0 - dist) * 0.5, 1.0).astype(np.float32)
    mwvec = (mask_b[2:L] * weight).astype(np.float32)  # [L-2]
    vals = np.zeros(L, dtype=np.float32)
    vals[: L - 2] = mwvec
    mw = vals.reshape(128, SEG)

    mt = mask_b.astype(np.float32).copy()
    mt[0:2] = 0.0
    maskt = mt.reshape(128, SEG)

    cst32 = np.concatenate([mw, maskt], axis=1)  # [128, 64]

    return {
        "states": np.ascontiguousarray(
            states_b.astype(ml_dtypes.bfloat16).reshape(128, SEG * D)
        ),
        "cst16": cst16,
        "cst32": np.ascontiguousarray(cst32),
    }


def _get_nc(split_mode="v3"):
    key = ("nc", split_mode)
    if key not in _CACHE:
        _CACHE[key] = _build_nc(split_mode)
    return _CACHE[key]


def _run(states, reasoning_mask, result_token_positions, trace=False,
         split_mode="v3"):
    from concourse.bass_utils import run_bass_kernel_spmd

    states = np.asarray(states, dtype=np.float32)
    mask = np.asarray(reasoning_mask, dtype=np.float32)
    rp = np.asarray(result_token_positions)

    cst16 = _host_consts()
    in_maps = [
        _per_core_inputs(states[b], mask[b], rp[b], cst16)
        for b in range(N_CORES)
    ]
    nc = _get_nc(split_mode)
    res = run_bass_kernel_spmd(
        nc, in_maps, core_ids=list(range(N_CORES)), trace=trace
    )
    partials = np.stack([res.results[i]["out"][0] for i in range(N_CORES)])  # [8, 2]
    s = partials[:, 0].astype(np.float64).sum()
    m = partials[:, 1].astype(np.float64).sum()
    value = np.float32(s / (m + 1e-9))
    return value, res


def kernel(states, reasoning_mask, result_token_positions):
    value, _ = _run(states, reasoning_mask, result_token_positions)
    return np.asarray(value, dtype=np.float32)
